# revision 59
# baseline (speedup 1.0000x reference)
"""MultiHeadDiffAttention Trainium2 kernel (8 NeuronCores).

Sharding: batch (4) x head-group (2 groups of 8 heads) = 8 cores.
Each core computes a partial (T, C) c_proj output for its batch element
restricted to its 8 heads; the host sums the two head-group partials per
batch element.

Per-core pipeline (fp16 matmuls on PE):
  1. Host supplies xT (C on partitions) and head-major-permuted weights
     (contiguous 2KB DMA lines per head slice). The x upload is split
     across the three DMA rings (sync/scalar/gpsimd) and by t-half; a
     single transfer tops out near ~95 GB/s, well below aggregate. wv/wc
     uploads are anchored behind real data deps so Tile cannot hoist them
     into the startup window and steal bandwidth from x.
  2. Projections: Q1/Q2 and K1/K2 in (head_dim, T) layout; V in (T, vdim)
     layout with an appended ones column.
  3. Scores: S^T(k,q) = K^T-tiles x Q^T per stream, contract d=64. The two
     streams sit in PE row-groups 0-63 / 64-127 and are emitted in
     contiguous 64-contract batches of 2 units (4 matmuls) so PE row
     tiling runs them concurrently; 128-contract work between batches
     would force a mode-switch drain and serialize them. exp via ScalarE
     (fused 1/8 scale) -> fp16 P, causal diag mask via gpsimd
     affine_select.
  4. PV: P-tile^T x [V|1] accumulated over k-tiles gives Y and the softmax
     denominator in one matmul. Stream combine fused into one DVE
     scalar_tensor_tensor: z = Y1 + gneg*Y2 with gneg = -lam*den1/den2
     (z = den1*(a1 - lam*a2) @ V; LayerNorm is scale-invariant per row, so
     normalizing z with eps scaled by den1^2 reproduces the reference).
  5. LN via bn_stats/bn_aggr + exp(-0.5*ln(var+eps*den1^2) + ln(1-li)).
  6. fp16 PE-transpose of y_ln, c_proj vs host-sliced fp16 Wc rows, with
     the transposes of tile i-1 emitted before tile i's PV so the PSUM
     evacuations hide under PE work; final tile streams out in 256-col
     chunks to overlap the last DMA drain.

Schedule: per head, the 6 score batches alternate with 128-contract
backlog work (PV of the previous head, Q/K projections of the NEXT head,
V projection during head 0) to hide the ScalarE exp latency (the real
per-head floor) behind PE work.

Note: fp8 was evaluated and rejected: e4m3 matmul noise (~5% RMS per
plain-cast stage) alone exceeds the 2e-2 gate, and hi-lo splitting
spends the DoubleRow speedup on precision (DR doubles contraction per
pass, not output rate, so 3-product hi-lo is slower than fp16).
"""

import contextlib
import ctypes
import math
import sys
import types

import numpy as np

sys.path.insert(0, "/opt/trn_rl_repo")


def _install_ntff_hook():
    """Provide antenv.axon_hooks if the image lacks it (for trace=True)."""
    try:
        from antenv.axon_hooks import get_axon_ntff_profile_hook  # noqa: F401

        return
    except ImportError:
        pass

    so_path = "/opt/axon/libaxon_pjrt.so"

    def _make_hook():
        try:
            lib = ctypes.CDLL(so_path)
        except OSError:
            return None
        if not hasattr(lib, "axon_start_nrt_profile"):
            return None
        lib.axon_start_nrt_profile.argtypes = [
            ctypes.POINTER(ctypes.c_int64),
            ctypes.c_size_t,
        ]
        lib.axon_start_nrt_profile.restype = ctypes.c_int64
        lib.axon_stop_nrt_profile.argtypes = [ctypes.c_char_p]
        lib.axon_stop_nrt_profile.restype = ctypes.c_int64

        @contextlib.contextmanager
        def _hook(output_dir, device_ids):
            import jax

            jax.devices()
            if device_ids:
                ids = (ctypes.c_int64 * len(device_ids))(*device_ids)
                rc = lib.axon_start_nrt_profile(ids, len(device_ids))
            else:
                rc = lib.axon_start_nrt_profile(None, 0)
            if rc != 0:
                raise RuntimeError(f"axon_start_nrt_profile rc={rc}")
            try:
                yield
            finally:
                n = lib.axon_stop_nrt_profile(str(output_dir).encode())
                if n < 0:
                    raise RuntimeError(f"axon_stop_nrt_profile rc={n}")

        return _hook

    mod = types.ModuleType("antenv.axon_hooks")
    _the_hook = _make_hook()
    mod.get_axon_ntff_profile_hook = lambda: _the_hook
    sys.modules["antenv.axon_hooks"] = mod


_install_ntff_hook()

import concourse.bass as bass  # noqa: E402
import concourse.mybir as mybir  # noqa: E402
import concourse.tile as tile  # noqa: E402
from concourse.masks import make_identity  # noqa: E402

P = 128
T = 1024
C = 1024
NH = 8  # heads per core
HS = 64
LAMBDA_INIT = 0.8 - 0.6 * math.exp(-0.3 * (2 - 1))
LN_EPS = 1e-5
N_CORES = 8

f32 = mybir.dt.float32
f16 = mybir.dt.float16
Alu = mybir.AluOpType
Act = mybir.ActivationFunctionType


def build_program():
    nc = bass.Bass()
    xt_d = nc.dram_tensor("xt", [C, T], f16, kind="ExternalInput")
    wq_d = nc.dram_tensor("wq", [C, C], f16, kind="ExternalInput")
    wk_d = nc.dram_tensor("wk", [C, C], f16, kind="ExternalInput")
    wv_d = nc.dram_tensor("wv", [C, C], f16, kind="ExternalInput")
    wc_d = nc.dram_tensor("wc", [C, C], f16, kind="ExternalInput")
    lamneg_d = nc.dram_tensor("lamneg", [P, NH], f32, kind="ExternalInput")
    out_d = nc.dram_tensor("out", [T, C], f16, kind="ExternalOutput")

    ln_bias = float(math.log(1.0 - LAMBDA_INIT))

    with tile.TileContext(nc) as tc:
        with (
            tc.tile_pool(name="const", bufs=1) as const,
            tc.tile_pool(name="ydata", bufs=8) as y_pool,
            tc.tile_pool(name="vdata", bufs=8) as v_p,
        ):
            ident = const.tile([P, P], f16, tag="ident")
            make_identity(nc, ident)
            lamneg = const.tile([P, NH], f32, tag="lamneg")
            den_store = const.tile([P, NH, 8], f32, tag="den")
            lnb = const.tile([P, 1], f32, tag="lnb")
            nc.vector.memset(lnb, ln_bias)

            wc_sb = [y_pool.tile([P, C], f16, tag="wc", name="wcsb") for _ in range(8)]
            y_tiles = [y_pool.tile([P, NH * P], f16, tag="y", name="yt") for _ in range(8)]
            mu_tiles = [y_pool.tile([P, NH], f32, tag="mu", name="mu") for _ in range(8)]
            var_tiles = [y_pool.tile([P, NH], f32, tag="var", name="var") for _ in range(8)]
            v_aug = [v_p.tile([P, NH, 132], f16, tag="v", name="vaug") for _ in range(8)]

            p_ctx = tc.tile_pool(name="pprob", bufs=4)
            p_pool = p_ctx.__enter__()
            small_ctx = tc.tile_pool(name="smallc", bufs=16)
            small = small_ctx.__enter__()

            def pv_mms(h, s, i, pcs, ypool):
                """PV matmuls for one (stream, q-tile): returns psum [P,129]."""
                n, t = i // 4, i % 4
                pch = pcs[(s, n)]
                yp = ypool.tile([P, 129], f32, tag="psY", name="yp")
                for j in range(i + 1):
                    nc.tensor.matmul(
                        yp,
                        lhsT=pch[:, j, 128 * t : 128 * (t + 1)],
                        rhs=v_aug[j][:, h, 0:129],
                        start=(j == 0),
                        stop=(j == i),
                    )
                return yp

            def pv_s0(h, i, pcs, ypool):
                yp = pv_mms(h, 0, i, pcs, ypool)
                nc.scalar.activation(
                    out=y_tiles[i][:, 128 * h : 128 * (h + 1)],
                    in_=yp[:, 0:128],
                    func=Act.Copy,
                )
                nc.vector.tensor_copy(
                    out=den_store[:, h, i : i + 1], in_=yp[:, 128:129]
                )

            def pv_s1(h, i, pcs, ypool):
                yp = pv_mms(h, 1, i, pcs, ypool)
                r2 = small.tile([P, 1], f32, tag="r2", name="r2")
                nc.vector.reciprocal(out=r2, in_=yp[:, 128:129])
                gneg = small.tile([P, 1], f32, tag="gneg", name="gneg")
                nc.vector.tensor_mul(
                    out=gneg, in0=den_store[:, h, i : i + 1], in1=r2
                )
                nc.vector.tensor_mul(out=gneg, in0=gneg, in1=lamneg[:, h : h + 1])
                ysl = y_tiles[i][:, 128 * h : 128 * (h + 1)]
                nc.vector.scalar_tensor_tensor(
                    out=ysl, in0=yp[:, 0:128], scalar=gneg, in1=ysl,
                    op0=Alu.mult, op1=Alu.add,
                )
                bs = small.tile([P, nc.vector.BN_STATS_DIM], f32, tag="bs", name="bs")
                nc.vector.bn_stats(out=bs, in_=ysl)
                mv = small.tile([P, nc.vector.BN_AGGR_DIM], f32, tag="mv", name="mv")
                nc.vector.bn_aggr(out=mv, in_=bs)
                nc.vector.tensor_copy(out=mu_tiles[i][:, h : h + 1], in_=mv[:, 0:1])
                nc.vector.tensor_copy(out=var_tiles[i][:, h : h + 1], in_=mv[:, 1:2])

            def ln_tile(i):
                """veps = var + eps*den1^2 -> invstd via Ln/Exp, then the
                per-head LN apply (both ACT functions live in the
                natural_log_exp table set: one switch total)."""
                d1 = den_store[:, :, i : i + 1].rearrange("p h one -> p (h one)")
                veps = small.tile([P, NH], f32, tag="veps", name="veps")
                nc.vector.tensor_mul(out=veps, in0=d1, in1=d1)
                nc.vector.tensor_scalar(
                    out=veps, in0=veps, scalar1=LN_EPS, scalar2=None,
                    op0=Alu.mult,
                )
                nc.vector.tensor_add(out=veps, in0=veps, in1=var_tiles[i])
                inv = small.tile([P, NH], f32, tag="inv", name="inv")
                nc.scalar.activation(out=inv, in_=veps, func=Act.Ln)
                nc.scalar.activation(
                    out=inv, in_=inv, func=Act.Exp, scale=-0.5, bias=lnb
                )
                for h in range(NH):
                    nc.vector.tensor_scalar(
                        out=y_tiles[i][:, 128 * h : 128 * (h + 1)],
                        in0=y_tiles[i][:, 128 * h : 128 * (h + 1)],
                        scalar1=mu_tiles[i][:, h : h + 1],
                        scalar2=inv[:, h : h + 1],
                        op0=Alu.subtract,
                        op1=Alu.mult,
                    )

            # ---------- merged projections + attention ----------
            _psum_ctxs = [
                tc.tile_pool(name="psB2", bufs=2, space="PSUM"),
                tc.tile_pool(name="psS", bufs=4, space="PSUM"),
                tc.tile_pool(name="psY", bufs=2, space="PSUM"),
            ]
            psB2, psS, psY = [c.__enter__() for c in _psum_ctxs]
            _sbuf_ctxs = [
                tc.tile_pool(name="xT", bufs=1),
                tc.tile_pool(name="wv", bufs=1),
                tc.tile_pool(name="wqk", bufs=2),
                tc.tile_pool(name="qk", bufs=2),
            ]
            xT_p, wv_p, wqk_p, qk_p = [c.__enter__() for c in _sbuf_ctxs]
            if True:
                xT_all = xT_p.tile([P, 8, T], f16, tag="xT", name="xT")
                xT = [xT_all[:, c, :] for c in range(8)]
                wv_all = wv_p.tile([P, 8, C], f16, tag="w", name="wsb")
                wv_sb = [wv_all[:, c, :] for c in range(8)]

                def emit_wdma(h):
                    """One contiguous-row DMA per weight matrix for head h.
                    Host pre-permutes so row 128h+p holds head h's (c, d)
                    block: out[p, c, d] = w_dma[128h + p, 128c + d]."""
                    tiles = []
                    for w_d, tag, nm in ((wq_d, "wq", "wqh"), (wk_d, "wk", "wkh")):
                        wt = wqk_p.tile([P, 8, P], f16, tag=tag, name=nm)
                        src_ap = w_d[128 * h : 128 * (h + 1), :].rearrange(
                            "p (c d) -> p c d", c=8
                        )
                        nc.sync.dma_start(out=wt, in_=src_ap)
                        tiles.append(wt)
                    return tiles

                def proj_half(wt, dest, n):
                    """(head_dim 128, 512) projection half."""
                    ps = psB2.tile([P, 512], f32, tag="psB2", name="pps")
                    for c in range(8):
                        nc.tensor.matmul(
                            ps,
                            lhsT=wt[:, c, :],
                            rhs=xT[c][:, 512 * n : 512 * (n + 1)],
                            start=(c == 0),
                            stop=(c == 7),
                        )
                    nc.vector.tensor_copy(
                        out=dest[:, 512 * n : 512 * (n + 1)], in_=ps
                    )

                def score_batch(h, qT, kT, pcs, units):
                    """Batch of score units: all matmuls first (contiguous
                    64-contract mode so the two streams' row-groups overlap),
                    then the exps + diag masks."""
                    sps = []
                    for (n, j) in units:
                        qlo = 128 * max(0, j - 4 * n)
                        sp2 = [
                            psS.tile([P, 512], f32, tag="psS", name="sp")
                            for _ in range(2)
                        ]
                        for s in range(2):
                            nc.tensor.matmul(
                                sp2[s][:, qlo:512],
                                lhsT=kT[64 * s : 64 * (s + 1), 128 * j : 128 * (j + 1)],
                                rhs=qT[
                                    64 * s : 64 * (s + 1),
                                    512 * n + qlo : 512 * (n + 1),
                                ],
                                start=True,
                                stop=True,
                            )
                        sps.append((n, j, qlo, sp2))
                    for (n, j, qlo, sp2) in sps:
                        t = j - 4 * n
                        for s in range(2):
                            pch = pcs[(s, n)]
                            nc.scalar.activation(
                                out=pch[:, j, qlo:512],
                                in_=sp2[s][:, qlo:512],
                                func=Act.Exp,
                                scale=0.125,
                            )
                            if 0 <= t <= 3:
                                nc.gpsimd.affine_select(
                                    out=pch[:, j, 128 * t : 128 * (t + 1)],
                                    in_=pch[:, j, 128 * t : 128 * (t + 1)],
                                    compare_op=Alu.is_ge,
                                    fill=0.0,
                                    base=0,
                                    pattern=[[1, 128]],
                                    channel_multiplier=-1,
                                )

                def vproj_unit(t, n):
                    """V-projection tile, interleaved into heads 0/1."""
                    ps = psB2.tile([P, 512], f32, tag="psB2", name="pps")
                    for c in range(8):
                        nc.tensor.matmul(
                            ps,
                            lhsT=xT[c][:, 128 * t : 128 * (t + 1)],
                            rhs=wv_sb[c][:, 512 * n : 512 * (n + 1)],
                            start=(c == 0),
                            stop=(c == 7),
                        )
                    if n == 0:
                        nc.vector.tensor_copy(
                            out=v_aug[t][:, 0:4, 0:128],
                            in_=ps.rearrange("p (g d) -> p g d", g=4),
                        )
                    else:
                        nc.scalar.activation(
                            out=v_aug[t][:, 4:8, 0:128],
                            in_=ps.rearrange("p (g d) -> p g d", g=4),
                            func=Act.Copy,
                        )
                    if n == 1:
                        nc.gpsimd.memset(v_aug[t][:, :, 128:129], 1.0)

                # PE warm-up: dummy matmuls with no input deps keep the
                # HAM clock ramping while the initial DMAs land
                dum = const.tile([P, 512], f16, tag="dum")
                nc.vector.memset(dum, 0.0)

                def emit_dummies(k, pool=psS):
                    for _ in range(k):
                        scr = pool.tile([P, 512], f32, tag="psS", name="sp")
                        for _ in range(8):
                            nc.tensor.matmul(
                                scr, lhsT=dum[:, 0:128], rhs=dum,
                                start=True, stop=True,
                            )

                emit_dummies(4)

                # critical path first: head-0 weights + xT split over the
                # sync/scalar queues; wv owns the gpsimd queue (needed later)
                wt0 = wqk_p.tile([P, 8, P], f16, tag="wq", name="wqh")
                nc.sync.dma_start(
                    out=wt0,
                    in_=wq_d[0:128, :].rearrange("p (c d) -> p c d", c=8),
                )
                wt1 = wqk_p.tile([P, 8, P], f16, tag="wk", name="wkh")
                nc.scalar.dma_start(
                    out=wt1,
                    in_=wk_d[0:128, :].rearrange("p (c d) -> p c d", c=8),
                )
                wts = [wt0, wt1]
                # three concurrent transfers (sync + scalar + gpsimd rings):
                # a single DMA transfer tops out near ~95 GB/s, well below
                # the aggregate bandwidth
                xt_r = xt_d.rearrange("(c p) t -> p c t", p=P)
                # t-half A first on every ring: the first projection halves
                # and all n=0 score units consume only t in [0, 512)
                for lo, hi, eng in ((0, 3, nc.sync), (3, 5, nc.scalar), (5, 8, nc.gpsimd)):
                    eng.dma_start(
                        out=xT_all[:, lo:hi, 0:512], in_=xt_r[:, lo:hi, 0:512]
                    )
                for lo, hi, eng in ((0, 3, nc.sync), (3, 5, nc.scalar), (5, 8, nc.gpsimd)):
                    eng.dma_start(
                        out=xT_all[:, lo:hi, 512:1024], in_=xt_r[:, lo:hi, 512:1024]
                    )
                # gate wv behind the x upload (real data dep so Tile
                # cannot hoist it): x is the critical path to the first
                # projections
                nc.gpsimd.tensor_copy(out=wv_sb[0][:, 0:1], in_=xT_all[:, 0, 0:1])
                for c in range(8):
                    nc.gpsimd.dma_start(
                        out=wv_sb[c], in_=wv_d[128 * c : 128 * (c + 1), :]
                    )
                nc.gpsimd.dma_start(out=lamneg, in_=lamneg_d[:, :])

                pcs_prev = None
                cur_qT = qk_p.tile([P, T], f16, tag="q", name="qT")
                cur_kT = qk_p.tile([P, T], f16, tag="k", name="kT")
                for n in range(2):
                    proj_half(wts[0], cur_qT, n)
                for n in range(2):
                    proj_half(wts[1], cur_kT, n)

                for h in range(NH):
                    qT, kT = cur_qT, cur_kT
                    if h == 6:
                        # real-data gate: Tile hoists dependency-free DMAs
                        # to t=0, which would steal startup DMA bandwidth
                        # from the x upload. A copy that depends on head-4's
                        # PV output anchors the wc upload to ~mid-kernel.
                        for d in range(8):
                            nc.gpsimd.tensor_copy(
                                out=wc_sb[d][:, 0:1], in_=mu_tiles[0][:, 4:5]
                            )
                            nc.gpsimd.dma_start(
                                out=wc_sb[d], in_=wc_d[128 * d : 128 * (d + 1), :]
                            )
                    if h + 1 < NH:
                        next_wts = emit_wdma(h + 1)
                        next_qT = qk_p.tile([P, T], f16, tag="q", name="qT")
                        next_kT = qk_p.tile([P, T], f16, tag="k", name="kT")
                    pcs = {
                        (s, n): p_pool.tile(
                            [P, 4 * n + 4, 512], f16,
                            tag=f"p{n}", name="pch", bufs=4,
                        )
                        for s in range(2)
                        for n in range(2)
                    }
                    # 128-contract backlog to interleave between score batches
                    backlog = []
                    if h == 0:
                        backlog += [("v", t, n) for t in range(8) for n in range(2)]
                    else:
                        for i in range(8):
                            backlog.append(("pv0", h - 1, i))
                            backlog.append(("pv1", h - 1, i))
                    if h + 1 < NH:
                        # mid-backlog, not last: the next head's first score
                        # batch must not wait on the final projection
                        # half's PSUM-evacuation CAST
                        mid = len(backlog) // 2
                        backlog[mid:mid] = [
                            ("projq", 0), ("projq", 1),
                            ("projk", 0), ("projk", 1),
                        ]

                    def run_item(u):
                        if u[0] == "v":
                            vproj_unit(u[1], u[2])
                        elif u[0] == "pv0":
                            pv_s0(u[1], u[2], pcs_prev, psY)
                        elif u[0] == "pv1":
                            pv_s1(u[1], u[2], pcs_prev, psY)
                        elif u[0] == "projq":
                            proj_half(next_wts[0], next_qT, u[1])
                        else:
                            proj_half(next_wts[1], next_kT, u[1])

                    # exp-load-balanced pairing; the cheapest pair goes
                    # LAST so its exps release the psum banks quickly for
                    # the next head's first batch
                    sunits = [
                        (0, 0), (0, 1), (0, 2), (1, 0), (1, 1), (1, 2),
                        (1, 3), (1, 4), (1, 5), (1, 6), (1, 7), (0, 3),
                    ]
                    batches = [sunits[k : k + 2] for k in range(0, len(sunits), 2)]
                    done = 0
                    for bi, batch in enumerate(batches):
                        score_batch(h, qT, kT, pcs, batch)
                        while done < len(backlog) and (bi + 1) * len(
                            backlog
                        ) >= (done + 1) * len(batches):
                            run_item(backlog[done])
                            done += 1
                    while done < len(backlog):
                        run_item(backlog[done])
                        done += 1
                    pcs_prev = pcs
                    if h + 1 < NH:
                        wts = next_wts
                        cur_qT, cur_kT = next_qT, next_kT

            # ---------- tail: PV(7) + LN + transpose pipeline, then c_proj ----------
            # the phase-A PSUM pools stay open (no pool-transition barrier);
            # only the big SBUF pools are released for the tail tiles
            for c in reversed(_sbuf_ctxs):
                c.__exit__(None, None, None)
            with (
                tc.tile_pool(name="ylnT", bufs=1) as ylnT_p,
                tc.tile_pool(name="outp", bufs=3) as out_p,
            ):
                ylnT = ylnT_p.tile([P, 8, T], f16, tag="ylnT", name="ylnT")

                def emit_trans(i):
                    for dh in range(2):
                        pt = psB2.tile([P, 512], f16, tag="psB2", name="pps")
                        for w in range(4):
                            dd = 4 * dh + w
                            nc.tensor.transpose(
                                out=pt[:, 128 * w : 128 * (w + 1)],
                                in_=y_tiles[i][:, 128 * dd : 128 * (dd + 1)],
                                identity=ident,
                            )
                        nc.vector.tensor_copy(
                            out=ylnT[:, 4 * dh : 4 * dh + 4, 128 * i : 128 * (i + 1)],
                            in_=pt.rearrange("p (g d) -> p g d", g=4),
                        )
                def emit_cproj(i, fine=False):
                    # c_proj row-block m = i only needs this q-tile's
                    # transposed columns
                    osb = out_p.tile([P, C], f16, tag="osb")
                    for n in range(2):
                        ps = psS.tile([P, 512], f32, tag="psS", name="sp")
                        for d in range(8):
                            nc.tensor.matmul(
                                ps,
                                lhsT=ylnT[:, d, 128 * i : 128 * (i + 1)],
                                rhs=wc_sb[d][:, 512 * n : 512 * (n + 1)],
                                start=(d == 0),
                                stop=(d == 7),
                            )
                        for q in range(2 if fine else 1):
                            w0 = 512 * n + 256 * q
                            w1 = w0 + (256 if fine else 512)
                            p0 = 256 * q
                            p1 = p0 + (256 if fine else 512)
                            nc.scalar.activation(
                                out=osb[:, w0:w1], in_=ps[:, p0:p1], func=Act.Copy
                            )
                            dma_eng = nc.sync if (n + q) % 2 == 0 else nc.scalar
                            dma_eng.dma_start(
                                out=out_d[128 * i : 128 * (i + 1), w0:w1],
                                in_=osb[:, w0:w1],
                            )

                tail_order = [2, 1, 0, 3, 7, 6, 5, 4]
                prev_i = None
                prev2_i = None
                for i in tail_order:
                    if prev_i is not None:
                        emit_trans(prev_i)
                    pv_s0(NH - 1, i, pcs_prev, psY)
                    pv_s1(NH - 1, i, pcs_prev, psY)
                    ln_tile(i)
                    if prev2_i is not None:
                        emit_cproj(prev2_i)
                    prev2_i = prev_i
                    prev_i = i
                emit_trans(prev_i)
                emit_cproj(prev2_i)
                emit_cproj(prev_i, fine=True)

            for c in reversed(_psum_ctxs):
                c.__exit__(None, None, None)
            small_ctx.__exit__(None, None, None)
            p_ctx.__exit__(None, None, None)

    bass._bass_rust.generate_event_semaphores(nc)
    return nc


_NC = None


def _get_program():
    global _NC
    if _NC is None:
        _NC = build_program()
    return _NC


def make_in_maps(inputs):
    """Host-side sharding: per-core input dicts."""
    x = np.ascontiguousarray(np.asarray(inputs["x"], dtype=np.float32))
    Wq1 = np.asarray(inputs["Wq1"], dtype=np.float32)
    Wq2 = np.asarray(inputs["Wq2"], dtype=np.float32)
    Wk1 = np.asarray(inputs["Wk1"], dtype=np.float32)
    Wk2 = np.asarray(inputs["Wk2"], dtype=np.float32)
    Wv = np.asarray(inputs["Wv"], dtype=np.float32)
    Wc = np.asarray(inputs["Wc"], dtype=np.float32)
    lq1 = np.asarray(inputs["lq1"], dtype=np.float32)
    lk1 = np.asarray(inputs["lk1"], dtype=np.float32)
    lq2 = np.asarray(inputs["lq2"], dtype=np.float32)
    lk2 = np.asarray(inputs["lk2"], dtype=np.float32)

    lam1 = np.exp(np.sum(lq1 * lk1, axis=-1))
    lam2 = np.exp(np.sum(lq2 * lk2, axis=-1))
    lam_full = (lam1 - lam2 + LAMBDA_INIT).astype(np.float32)  # (16,)

    in_maps = []
    for core in range(N_CORES):
        b, hg = core // 2, core % 2
        heads = np.arange(NH) + NH * hg  # global head idx
        wq = np.empty((C, C), np.float32)
        wk = np.empty((C, C), np.float32)
        wv = np.empty((C, C), np.float32)
        for h in range(NH):
            H = NH * hg + h
            wq[:, 128 * h : 128 * h + 64] = Wq1[:, HS * H : HS * (H + 1)]
            wq[:, 128 * h + 64 : 128 * (h + 1)] = Wq2[:, HS * H : HS * (H + 1)]
            wk[:, 128 * h : 128 * h + 64] = Wk1[:, HS * H : HS * (H + 1)]
            wk[:, 128 * h + 64 : 128 * (h + 1)] = Wk2[:, HS * H : HS * (H + 1)]
            wv[:, 128 * h : 128 * (h + 1)] = Wv[:, 128 * H : 128 * (H + 1)]
        wc = np.ascontiguousarray(Wc[1024 * hg : 1024 * (hg + 1), :])
        lamneg = np.broadcast_to(
            -lam_full[heads][None, :], (P, NH)
        ).astype(np.float32)
        def perm_head_major(w):
            # [128c+p, 128h+d] -> [128h+p, 128c+d]
            return np.ascontiguousarray(
                w.reshape(8, P, 8, P).transpose(2, 1, 0, 3).reshape(C, C)
            )

        in_maps.append(
            {
                "xt": np.ascontiguousarray(x[b].T.astype(np.float16)),
                "wq": perm_head_major(wq).astype(np.float16),
                "wk": perm_head_major(wk).astype(np.float16),
                "wv": wv.astype(np.float16),
                "wc": wc.astype(np.float16),
                "lamneg": np.ascontiguousarray(lamneg),
            }
        )
    return in_maps


def run(inputs, trace=False, **kw):
    from concourse.bass_utils import run_bass_kernel_spmd

    nc = _get_program()
    in_maps = make_in_maps(inputs)
    res = run_bass_kernel_spmd(
        nc, in_maps, core_ids=list(range(N_CORES)), trace=trace, **kw
    )
    B = 4
    out = np.empty((B, T, C), np.float32)
    for b in range(B):
        out[b] = res.results[2 * b]["out"].astype(np.float32) + res.results[
            2 * b + 1
        ]["out"].astype(np.float32)
    return out, res


def kernel(**inputs) -> np.ndarray:
    out, _ = run(inputs, trace=False)
    return out
